# revision 3
# baseline (speedup 1.0000x reference)
"""Trainium2 Bass kernel for nn_ContrastiveLoss (N=384, D=128, 8 cores).

Math restructure (validated against the reference):
  denom[i,p] = G[i,p] + C_i,  G[i,p] = sum_j g[i,j]*[a_ij < a_ip],
  g = (POS_W-1)*u - NEG_W*v,  C_i = sum_j (u + NEG_W*v) = sum_j w_off  (NEG_W=1),
  loss = -(sum_i ssum_i - sum_{i,p!=i} ln denom[i,p]) / (N(N-1)).

Key trick: [a_ij < a_ip] <=> r_j < r_p with r_x = (y_x - 2*y_i)*y_x (the
y_i^2 terms cancel), so each anchor i needs only its transformed row
r^i_p over all p (fp16) plus per-(i, j-chunk) threshold/weight columns
r_j / g_j as per-partition scalars.

Engine mapping (per core, 48 anchors x 3 j-chunks):
 - all 48 r-rows are computed in ONE fused DVE scalar_tensor_tensor as a
   [48, 384] matrix, bounced through DRAM, and partition-broadcast into a
   [128, 48*384] SBUF buffer by 8 chunked DMAs (DMA engines are otherwise
   idle; this removes all per-anchor broadcast work from compute engines);
 - one fused DVE tensor_scalar (is_gt, then mult by g_j) per (i, chunk)
   produces the masked-g tile in fp16 (~368 ns; the DVE never fast-modes
   tensor_scalar on HW, so fp16 costs nothing extra and fp8 is slower);
 - the PE reduces over j with an all-ones fp16 [128, 32] lhsT, writing
   replicated 32-row blocks so three anchors pack per PSUM bank at
   partitions {0, 32, 64} with per-block accumulation groups;
 - one Act Ln per bank (per-partition bias C_i via crepquad, accum_out)
   performs denom-add + log + row-sum over p in a single instruction;
   Act tables are kept to 3 loads by grouping Sqrt/Sigmoid/Exp+Ln phases.
Host packs inputs into one [128, PW] tensor (one input DMA) and reduces the
per-core [2, R] + [96, NG] outputs to the scalar loss.
"""

import os
import sys

import numpy as np

for _p in ("/opt/trn_rl_repo", "/root/.axon_site/_ro/trn_rl_repo"):
    if os.path.isdir(_p) and _p not in sys.path:
        sys.path.insert(0, _p)

import concourse.bass as bass
import concourse.bacc as bacc
import concourse.mybir as mybir
from concourse import tile
from concourse.bass_utils import run_bass_kernel_spmd

F32 = mybir.dt.float32
F16 = mybir.dt.float16
AF = mybir.ActivationFunctionType
OP = mybir.AluOpType

B = 192          # batch
N = 2 * B        # 384 rows/cols
D = 128          # embedding dim
NC = 8           # cores
R = N // NC      # 48 anchor rows per core
CH = N // 128    # 3 chunks of j / p
GS = 3           # G rows per PSUM bank (base partitions 0/32/64)
NG = R // GS     # 16 groups

TEMP = 2.0
TAU = 1.0
POS_W = 0.1
NEG_W = 1.0

# packed layout (fp32 columns)
C_ZT = 0
C_ZTOWN = C_ZT + N            # 384
C_YOWN = C_ZTOWN + R          # 432
C_OIDX = C_YOWN + R           # 480
C_YCOL = C_OIDX + R           # 528
C_JCOL = C_YCOL + CH          # 531
C_YBC = C_JCOL + CH           # 534
C_SEL = C_YBC + N             # 918  (rows 0:R)
C_E = C_SEL + 128             # 1046 (rows 0:R)
C_YOC = C_E + NG              # 1062 (rows 0:R, yowncol)
PW = C_YOC + 1                # 1063



def _build_program():
    nc = bacc.Bacc("TRN2", target_bir_lowering=False, debug=False, num_devices=NC)

    packed = nc.dram_tensor("packed", [128, PW], F32, kind="ExternalInput").ap()
    out1 = nc.dram_tensor("out1", [2, R], F32, kind="ExternalOutput").ap()
    out2 = nc.dram_tensor("out2", [96, NG], F32, kind="ExternalOutput").ap()
    q48d = nc.dram_tensor("q48d", [1, R * N], F16, kind="Internal").ap()

    with tile.TileContext(nc) as tc:
        with (
            tc.tile_pool(name="big", bufs=1) as big,
            tc.tile_pool(name="small", bufs=1) as small,
            tc.tile_pool(name="work", bufs=2) as work,
            tc.tile_pool(name="ts", bufs=8) as tpool,
            tc.tile_pool(name="lnout", bufs=2) as lnpool,
            tc.tile_pool(name="ps_pre", bufs=1, space="PSUM") as ps_pre,
            tc.tile_pool(name="ps_gt", bufs=2, space="PSUM") as ps_gt,
            tc.tile_pool(name="ps_acc", bufs=1, space="PSUM") as ps_acc,
            tc.tile_pool(name="ps_g", bufs=3, space="PSUM") as ps_g,
        ):
            # ---------- load input (ONE DMA) ----------
            pk = big.tile([128, PW], F32, tag="packed")
            nc.sync.dma_start(pk[:], packed)
            zT_s = pk[:, C_ZT:C_ZT + N]
            zTown_s = pk[:, C_ZTOWN:C_ZTOWN + R]
            yownrep = pk[:, C_YOWN:C_YOWN + R]
            ownidxrep = pk[:, C_OIDX:C_OIDX + R]
            ycolc = pk[:, C_YCOL:C_YCOL + CH]
            jcolc = pk[:, C_JCOL:C_JCOL + CH]
            ybcf32 = pk[:, C_YBC:C_YBC + N]
            sel_s = pk[0:R, C_SEL:C_SEL + 128]
            e_s = pk[0:R, C_E:C_E + NG]
            yowncol = pk[0:R, C_YOC:C_YOC + 1]

            # ---------- constants ----------
            ones128 = small.tile([128, 1], F32, tag="ones128")
            nc.vector.memset(ones128[:], 1.0)
            onesrow = small.tile([1, 128], F32, tag="onesrow")
            nc.vector.memset(onesrow[:], 1.0)
            ones32h = small.tile([128, 32], F16, tag="ones32h")
            nc.vector.memset(ones32h[:], 1.0)

            # ---------- casts / laundering ----------
            ybc16 = small.tile([128, N], F16, tag="ybc16")
            nc.gpsimd.tensor_copy(ybc16[:], ybcf32)
            ycolc16 = small.tile([128, CH], F16, tag="ycolc16")
            nc.gpsimd.tensor_copy(ycolc16[:], ycolc)
            y2owncol = small.tile([R, 1], F32, tag="y2owncol")
            nc.vector.tensor_tensor(y2owncol[:], yowncol, yowncol, op=OP.add)
            # q48[i, p] = (fl16(y_p) - 2*y_i) * fl16(y_p), one fused stt
            q48 = small.tile([R, N], F16, tag="q48")
            nc.vector.scalar_tensor_tensor(
                q48[:], ybc16[0:R, :], y2owncol[:, 0:1], ybc16[0:R, :],
                op0=OP.subtract, op1=OP.mult,
            )
            nc.sync.dma_start(q48d[0:1, :].rearrange("a (p f) -> a p f", p=R, f=N), q48[:])
            # broadcast rows of q48 across all 128 partitions: 8 chunked DMAs
            qall = big.tile([128, R * N], F16, tag="qall")
            QCH = 8
            qw = R * N // QCH
            for k in range(QCH):
                nc.sync.dma_start(
                    qall[:, k * qw:(k + 1) * qw],
                    q48d[0:1, k * qw:(k + 1) * qw].to_broadcast((128, qw)),
                )

            # ---------- squared norms ----------
            zsq = big.tile([128, N], F32, tag="zsq")
            nc.gpsimd.tensor_tensor(zsq[:], zT_s, zT_s, op=OP.mult)
            zsqown = small.tile([128, R], F32, tag="zsqown")
            nc.gpsimd.tensor_tensor(zsqown[:], zTown_s, zTown_s, op=OP.mult)

            n2own_ps = ps_pre.tile([1, R], F32, tag="pre")
            nc.tensor.matmul(n2own_ps[:], ones128[:], zsqown[:], start=True, stop=True)
            n2own_s = small.tile([1, R], F32, tag="n2own_s")
            nc.vector.tensor_copy(n2own_s[:], n2own_ps[:])
            n2ownrep_ps = ps_pre.tile([128, R], F32, tag="pre")
            nc.tensor.matmul(n2ownrep_ps[:], onesrow[:], n2own_s[:], start=True, stop=True)
            n2ownrep = small.tile([128, R], F32, tag="n2ownrep")
            nc.vector.tensor_copy(n2ownrep[:], n2ownrep_ps[:])

            n2colc = small.tile([128, CH], F32, tag="n2colc")
            for c in range(CH):
                n2c_ps = ps_pre.tile([128, 1], F32, tag="pre")
                nc.tensor.matmul(
                    n2c_ps[:], zsq[:, c * 128:(c + 1) * 128], ones128[:],
                    start=True, stop=True,
                )
                nc.vector.tensor_copy(n2colc[:, c:c + 1], n2c_ps[:])

            # ---------- per-chunk prep (pass 1: no Sigmoid/Exp yet) ----------
            gtc32 = small.tile([128, CH * R], F32, tag="gtc32")
            rtc16 = small.tile([128, CH * R], F16, tag="rtc16")
            # fp32 copy of the fp16-rounded thresholds (tensor_scalar wants
            # fp32 comparison scalars; values stay exactly fp16-representable
            # so ties against fp16 qall values resolve exactly)
            rtc32 = small.tile([128, CH * R], F32, tag="rtc32")
            atc_l, distt_l, samet_l, ndt_l = [], [], [], []
            for c in range(CH):
                csl = slice(c * R, (c + 1) * R)
                ycb = ycolc[:, c:c + 1].to_broadcast((128, R))
                ycb16 = ycolc16[:, c:c + 1].to_broadcast((128, R))

                gt_ps = ps_gt.tile([128, R], F32, tag="gt")
                nc.tensor.matmul(
                    gt_ps[:], zT_s[:, c * 128:(c + 1) * 128], zTown_s,
                    start=True, stop=True,
                )
                sqt = work.tile([128, R], F32, tag="sqt")
                nc.vector.scalar_tensor_tensor(
                    sqt[:], gt_ps[:], -2.0, n2ownrep[:], op0=OP.mult, op1=OP.add
                )
                nc.vector.tensor_tensor(
                    sqt[:], sqt[:], n2colc[:, c:c + 1].to_broadcast((128, R)), op=OP.add
                )
                sqr = work.tile([128, R], F32, tag="sqr")
                nc.vector.tensor_scalar(sqr[:], sqt[:], 0.0, None, op0=OP.max)
                distt = small.tile([128, R], F32, tag=f"distt{c}")
                distt_l.append(distt)
                nc.scalar.activation(distt[:], sqr[:], AF.Sqrt)

                atcraw = work.tile([128, R], F32, tag="atcraw")
                nc.vector.tensor_tensor(atcraw[:], yownrep, ycb, op=OP.subtract)
                atc = small.tile([128, R], F32, tag=f"atc{c}")
                atc_l.append(atc)
                nc.scalar.activation(atc[:], atcraw[:], AF.Abs)
                samet = small.tile([128, R], F32, tag=f"samet{c}")
                samet_l.append(samet)
                nc.vector.tensor_tensor(samet[:], yownrep, ycb, op=OP.is_lt)
                ndt = small.tile([128, R], F32, tag=f"ndt{c}")
                ndt_l.append(ndt)
                nc.vector.tensor_tensor(
                    ndt[:], ownidxrep, jcolc[:, c:c + 1].to_broadcast((128, R)),
                    op=OP.not_equal,
                )

                # r-form thresholds: r_j = (y16_j - 2*y_i)*y16_j
                t_r = work.tile([128, R], F32, tag="t_r")
                nc.vector.scalar_tensor_tensor(
                    t_r[:], yownrep, -2.0, ycb16, op0=OP.mult, op1=OP.add
                )
                nc.vector.tensor_tensor(rtc16[:, csl], t_r[:], ycb16, op=OP.mult)
                nc.vector.tensor_copy(rtc32[:, csl], rtc16[:, csl])

            # ---------- pass 2: Sigmoid ----------
            dwt_l = []
            for c in range(CH):
                dwt = small.tile([128, R], F32, tag=f"dwt{c}")
                dwt_l.append(dwt)
                nc.scalar.activation(dwt[:], atc_l[c][:], AF.Sigmoid, scale=TAU)

            # ---------- pass 3: Exp + weights + row sums + g ----------
            cs_ps = ps_acc.tile([1, 2 * R], F32, tag="acc")
            for c in range(CH):
                csl = slice(c * R, (c + 1) * R)
                et = work.tile([128, R], F32, tag="et")
                nc.scalar.activation(et[:], distt_l[c][:], AF.Exp, scale=-1.0 / TEMP)
                wt = work.tile([128, R], F32, tag="wt")
                nc.vector.tensor_tensor(wt[:], et[:], dwt_l[c][:], op=OP.mult)
                csin = work.tile([128, 2 * R], F32, tag="csin")
                nc.vector.tensor_tensor(csin[:, 0:R], wt[:], ndt_l[c][:], op=OP.mult)
                nc.vector.tensor_tensor(
                    csin[:, R:2 * R], distt_l[c][:], ndt_l[c][:], op=OP.mult
                )
                nc.tensor.matmul(
                    cs_ps[:], ones128[:], csin[:], start=(c == 0), stop=(c == CH - 1)
                )
                # g = wd0 * ((POS_W-1+NEG_W)*same - NEG_W)
                ga = work.tile([128, R], F32, tag="ga")
                nc.vector.tensor_scalar(
                    ga[:], samet_l[c][:], POS_W - 1.0 + NEG_W, NEG_W,
                    op0=OP.mult, op1=OP.subtract,
                )
                nc.vector.tensor_tensor(gtc32[:, csl], ga[:], csin[:, 0:R], op=OP.mult)

            cs_s = small.tile([1, 2 * R], F32, tag="cs_s")
            nc.vector.tensor_copy(cs_s[:], cs_ps[:])

            # ---------- crepquad: bias column per group ----------
            # Ccol[i] = C_i as a [R,1] column (PE transpose of the cs row)
            ccol_ps = ps_pre.tile([R, 1], F32, tag="pre")
            nc.tensor.matmul(
                ccol_ps[:], cs_s[0:1, 0:R], onesrow[0:1, 0:1], start=True, stop=True
            )
            ccol = small.tile([R, 1], F32, tag="ccol")
            nc.vector.tensor_copy(ccol[:], ccol_ps[:])
            # rhsEC[i,g] = C_i * [i//4 == g]
            rhs_ec = small.tile([R, NG], F32, tag="rhs_ec")
            nc.vector.tensor_scalar(rhs_ec[:], e_s, ccol[:, 0:1], None, op0=OP.mult)
            # crepquad[q,g] = sum_i SEL[i,q] * rhsEC[i,g] = C_{4g + q//32}
            cq_ps = ps_pre.tile([128, NG], F32, tag="pre")
            nc.tensor.matmul(cq_ps[:], sel_s, rhs_ec[:], start=True, stop=True)
            crepquad = small.tile([128, NG], F32, tag="crepquad")
            nc.vector.tensor_copy(crepquad[:], cq_ps[:])

            # ---------- main loop ----------
            lnaccg = small.tile([128, NG], F32, tag="lnaccg")
            for g in range(NG):
                gb = ps_g.tile([128, N], F32, tag="g")
                for k in range(GS):
                    i = GS * g + k
                    qrep = qall[:, i * N:(i + 1) * N]
                    thr = rtc32
                    for c in range(CH):
                        tso = tpool.tile([128, N], F16, tag="tso")
                        nc.vector.tensor_scalar(
                            tso[:], qrep,
                            thr[:, c * R + i:c * R + i + 1],
                            gtc32[:, c * R + i:c * R + i + 1],
                            op0=OP.is_gt, op1=OP.mult,
                        )
                        nc.tensor.matmul(
                            gb[32 * k:32 * k + 32, :], ones32h[:], tso[:],
                            start=(c == 0),
                            stop=(c == CH - 1),
                            skip_group_check=True,
                        )
                lnout = lnpool.tile([128, N], F32, tag="lnout")
                nc.scalar.activation(
                    lnout[0:96, :], gb[0:96, :], AF.Ln,
                    bias=crepquad[0:96, g:g + 1],
                    accum_out=lnaccg[0:96, g:g + 1],
                )

            # ---------- outputs ----------
            ssum = small.tile([1, R], F32, tag="ssum")
            nc.vector.tensor_scalar(
                ssum[:], cs_s[0:1, R:2 * R], -1.0 / TEMP, None, op0=OP.mult
            )
            lnc = small.tile([1, R], F32, tag="lnc")
            nc.scalar.activation(lnc[:], cs_s[0:1, 0:R], AF.Ln)
            nc.sync.dma_start(out1[0:1, :], ssum[:])
            nc.sync.dma_start(out1[1:2, :], lnc[:])
            nc.sync.dma_start(out2, lnaccg[0:96, :])

    nc.compile()
    return nc


_NC_CACHE = None


def _get_nc():
    global _NC_CACHE
    if _NC_CACHE is None:
        _NC_CACHE = _build_program()
    return _NC_CACHE


def _make_in_maps(embeddings, targets):
    emb = np.ascontiguousarray(np.asarray(embeddings, dtype=np.float32))
    tgt = np.ascontiguousarray(np.asarray(targets, dtype=np.float32))
    z = emb.transpose(1, 0, 2).reshape(N, D)
    zT = np.ascontiguousarray(z.T)                       # [D, N]
    y = np.concatenate([tgt, tgt], axis=0)[:, 0]         # [N]
    jidx = np.arange(N, dtype=np.float32)
    # constant indicator matrices for the crepquad build
    sel = (np.arange(R)[:, None] % GS == np.arange(128)[None, :] // 32)
    emat = (np.arange(R)[:, None] // GS == np.arange(NG)[None, :])
    in_maps = []
    for core in range(NC):
        sl = slice(core * R, (core + 1) * R)
        p = np.zeros((128, PW), np.float32)
        p[:, C_ZT:C_ZT + N] = zT
        p[:, C_ZTOWN:C_ZTOWN + R] = zT[:, sl]
        p[:, C_YOWN:C_YOWN + R] = y[None, sl]
        p[:, C_OIDX:C_OIDX + R] = jidx[None, sl]
        p[:, C_YCOL:C_YCOL + CH] = y.reshape(CH, 128).T
        p[:, C_JCOL:C_JCOL + CH] = jidx.reshape(CH, 128).T
        p[:, C_YBC:C_YBC + N] = y[None, :]
        p[0:R, C_SEL:C_SEL + 128] = sel
        p[0:R, C_E:C_E + NG] = emat
        p[0:R, C_YOC] = y[sl]
        in_maps.append({"packed": p})
    return in_maps


def _reduce_outs(outs_list):
    tot_s = 0.0
    tot_logd = 0.0
    ks = np.arange(R) % GS
    gs = np.arange(R) // GS
    for o in outs_list:
        o1 = np.asarray(o["out1"], dtype=np.float64)
        o2 = np.asarray(o["out2"], dtype=np.float64)
        tot_s += o1[0, :].sum()
        lnacc_i = o2[32 * ks, gs]        # [R] per-anchor sum_p ln(denom)
        tot_logd += (lnacc_i - o1[1, :]).sum()
    loss = -(tot_s - tot_logd) / (N * (N - 1))
    return np.float32(loss)


def _run(embeddings, targets, trace=False, **kw):
    nc = _get_nc()
    in_maps = _make_in_maps(embeddings, targets)
    res = run_bass_kernel_spmd(nc, in_maps, list(range(NC)), trace=trace, **kw)
    outs = [res.results[c] for c in range(NC)]
    return _reduce_outs(outs), res


def kernel(embeddings, targets):
    loss, _ = _run(embeddings, targets, trace=False)
    return loss


# revision 6
# speedup vs baseline: 1.2557x; 1.2557x over previous
"""Trainium2 Bass kernel for nn_ContrastiveLoss (N=384, D=128, 8 cores) — v2.

Math (same restructure as v1, new engine mapping):
  denom[i,p] = G[i,p] + C_i,  G[i,p] = sum_j g[i,j]*[a_ij < a_ip],
  g = (POS_W-1)*u - NEG_W*v,  C_i = sum_j (u + NEG_W*v) = sum_j w_off  (NEG_W=1),
  loss = -(sum_i ssum_i - sum_{i,p!=i} ln denom) / (N(N-1)).

Key trick: [a_ij < a_ip] <=> r_j < r_p with r_x = (y_x - 2*y_i)*y_x
(the y_i^2 terms cancel), so the comparison needs only a per-i transformed
row r (fp16, one scalar_tensor_tensor build) plus per-(i,chunk) columns
r_j / g_j as per-partition scalars.  One fused fp16 tensor_scalar
(is_gt, then mult by g_j, 4x DVE mode) per (i, j-chunk) yields the
masked-g tile; the PE reduces over j with an all-ones fp16 [128,1] lhsT
into G rows packed 4-per-PSUM-bank at partitions {0,32,64,96}.  One Act
Ln per bank (bias = C_i per 32-partition block via crepquad, accum_out)
does denom-add + log + row-sum in a single instruction.
"""

import os
import sys

import numpy as np

for _p in ("/opt/trn_rl_repo", "/root/.axon_site/_ro/trn_rl_repo"):
    if os.path.isdir(_p) and _p not in sys.path:
        sys.path.insert(0, _p)

import concourse.bass as bass
import concourse.bacc as bacc
import concourse.mybir as mybir
from concourse import tile
from concourse.bass_utils import run_bass_kernel_spmd

F32 = mybir.dt.float32
F16 = mybir.dt.float16
AF = mybir.ActivationFunctionType
OP = mybir.AluOpType

B = 192          # batch
N = 2 * B        # 384 rows/cols
D = 128          # embedding dim
NC = 8           # cores
R = N // NC      # 48 anchor rows per core
CH = N // 128    # 3 chunks of j / p
GS = 3           # G rows per PSUM bank (base partitions 0/32/64)
NG = R // GS     # 16 groups

TEMP = 2.0
TAU = 1.0
POS_W = 0.1
NEG_W = 1.0

# anchors whose compares run on the Act engine via Sign(r_p - r_j)
SIGNI = [i for i in range(48) if i % 3 == 1 and (i // 3) % 2 == 0]

# packed layout (fp32 columns)
C_ZT = 0
C_ZTOWN = C_ZT + N            # 384
C_YOWN = C_ZTOWN + R          # 432
C_OIDX = C_YOWN + R           # 480
C_YCOL = C_OIDX + R           # 528
C_JCOL = C_YCOL + CH          # 531
C_YBC = C_JCOL + CH           # 534
C_SEL = C_YBC + N             # 918  (rows 0:R)
C_E = C_SEL + 128             # 1046 (rows 0:R)
C_YOC = C_E + NG              # 1062 (rows 0:R, yowncol)
C_E2 = C_YOC + 1              # 1063 (rows 0:R, sign-anchor group mask)
PW = C_E2 + NG                # 1079



def _build_program():
    nc = bacc.Bacc("TRN2", target_bir_lowering=False, debug=False, num_devices=NC)

    packed = nc.dram_tensor("packed", [128, PW], F32, kind="ExternalInput").ap()
    out1 = nc.dram_tensor("out1", [2, R], F32, kind="ExternalOutput").ap()
    out2 = nc.dram_tensor("out2", [96, NG], F32, kind="ExternalOutput").ap()
    q48d = nc.dram_tensor("q48d", [1, R * N], F16, kind="Internal").ap()

    with tile.TileContext(nc) as tc:
        with (
            tc.tile_pool(name="big", bufs=1) as big,
            tc.tile_pool(name="small", bufs=1) as small,
            tc.tile_pool(name="work", bufs=2) as work,
            tc.tile_pool(name="ts", bufs=8) as tpool,
            tc.tile_pool(name="lnout", bufs=2) as lnpool,
            tc.tile_pool(name="ps_pre", bufs=1, space="PSUM") as ps_pre,
            tc.tile_pool(name="ps_gt", bufs=2, space="PSUM") as ps_gt,
            tc.tile_pool(name="ps_acc", bufs=1, space="PSUM") as ps_acc,
            tc.tile_pool(name="ps_g", bufs=3, space="PSUM") as ps_g,
        ):
            # ---------- load input (ONE DMA) ----------
            pk = big.tile([128, PW], F32, tag="packed")
            nc.sync.dma_start(pk[:], packed)
            zT_s = pk[:, C_ZT:C_ZT + N]
            zTown_s = pk[:, C_ZTOWN:C_ZTOWN + R]
            yownrep = pk[:, C_YOWN:C_YOWN + R]
            ownidxrep = pk[:, C_OIDX:C_OIDX + R]
            ycolc = pk[:, C_YCOL:C_YCOL + CH]
            jcolc = pk[:, C_JCOL:C_JCOL + CH]
            ybcf32 = pk[:, C_YBC:C_YBC + N]
            sel_s = pk[0:R, C_SEL:C_SEL + 128]
            e_s = pk[0:R, C_E:C_E + NG]
            yowncol = pk[0:R, C_YOC:C_YOC + 1]
            e2_s = pk[0:R, C_E2:C_E2 + NG]

            # ---------- constants ----------
            ones128 = small.tile([128, 1], F32, tag="ones128")
            nc.vector.memset(ones128[:], 1.0)
            onesrow = small.tile([1, 128], F32, tag="onesrow")
            nc.vector.memset(onesrow[:], 1.0)
            ones32h = small.tile([128, 32], F16, tag="ones32h")
            nc.vector.memset(ones32h[:], 1.0)

            # ---------- casts / laundering ----------
            ybc16 = small.tile([128, N], F16, tag="ybc16")
            nc.gpsimd.tensor_copy(ybc16[:], ybcf32)
            ycolc16 = small.tile([128, CH], F16, tag="ycolc16")
            nc.gpsimd.tensor_copy(ycolc16[:], ycolc)
            y2owncol = small.tile([R, 1], F32, tag="y2owncol")
            nc.vector.tensor_tensor(y2owncol[:], yowncol, yowncol, op=OP.add)
            # q48[i, p] = (fl16(y_p) - 2*y_i) * fl16(y_p), one fused stt
            q48 = small.tile([R, N], F16, tag="q48")
            nc.vector.scalar_tensor_tensor(
                q48[:], ybc16[0:R, :], y2owncol[:, 0:1], ybc16[0:R, :],
                op0=OP.subtract, op1=OP.mult,
            )
            nc.sync.dma_start(q48d[0:1, :].rearrange("a (p f) -> a p f", p=R, f=N), q48[:])
            # broadcast rows of q48 across all 128 partitions: 8 chunked DMAs
            qall = big.tile([128, R * N], F16, tag="qall")
            QCH = 8
            qw = R * N // QCH
            for k in range(QCH):
                nc.sync.dma_start(
                    qall[:, k * qw:(k + 1) * qw],
                    q48d[0:1, k * qw:(k + 1) * qw].to_broadcast((128, qw)),
                )

            # ---------- squared norms ----------
            zsq = big.tile([128, N], F32, tag="zsq")
            nc.gpsimd.tensor_tensor(zsq[:], zT_s, zT_s, op=OP.mult)
            zsqown = small.tile([128, R], F32, tag="zsqown")
            nc.gpsimd.tensor_tensor(zsqown[:], zTown_s, zTown_s, op=OP.mult)

            n2own_ps = ps_pre.tile([1, R], F32, tag="pre")
            nc.tensor.matmul(n2own_ps[:], ones128[:], zsqown[:], start=True, stop=True)
            n2own_s = small.tile([1, R], F32, tag="n2own_s")
            nc.vector.tensor_copy(n2own_s[:], n2own_ps[:])
            n2ownrep_ps = ps_pre.tile([128, R], F32, tag="pre")
            nc.tensor.matmul(n2ownrep_ps[:], onesrow[:], n2own_s[:], start=True, stop=True)
            n2ownrep = small.tile([128, R], F32, tag="n2ownrep")
            nc.vector.tensor_copy(n2ownrep[:], n2ownrep_ps[:])

            n2colc = small.tile([128, CH], F32, tag="n2colc")
            for c in range(CH):
                n2c_ps = ps_pre.tile([128, 1], F32, tag="pre")
                nc.tensor.matmul(
                    n2c_ps[:], zsq[:, c * 128:(c + 1) * 128], ones128[:],
                    start=True, stop=True,
                )
                nc.vector.tensor_copy(n2colc[:, c:c + 1], n2c_ps[:])

            # ---------- per-chunk prep (pass 1: no Sigmoid/Exp yet) ----------
            gtc32 = small.tile([128, CH * R], F32, tag="gtc32")
            rtc16 = small.tile([128, CH * R], F16, tag="rtc16")
            # fp32 copy of the fp16-rounded thresholds (tensor_scalar wants
            # fp32 comparison scalars; values stay exactly fp16-representable
            # so ties against fp16 qall values resolve exactly)
            rtc32 = small.tile([128, CH * R], F32, tag="rtc32")
            atc_l, distt_l, samet_l, ndt_l = [], [], [], []
            for c in range(CH):
                csl = slice(c * R, (c + 1) * R)
                ycb = ycolc[:, c:c + 1].to_broadcast((128, R))
                ycb16 = ycolc16[:, c:c + 1].to_broadcast((128, R))

                gt_ps = ps_gt.tile([128, R], F32, tag="gt")
                nc.tensor.matmul(
                    gt_ps[:], zT_s[:, c * 128:(c + 1) * 128], zTown_s,
                    start=True, stop=True,
                )
                sqt = work.tile([128, R], F32, tag="sqt")
                nc.vector.scalar_tensor_tensor(
                    sqt[:], gt_ps[:], -2.0, n2ownrep[:], op0=OP.mult, op1=OP.add
                )
                nc.vector.tensor_tensor(
                    sqt[:], sqt[:], n2colc[:, c:c + 1].to_broadcast((128, R)), op=OP.add
                )
                sqr = work.tile([128, R], F32, tag="sqr")
                nc.vector.tensor_scalar(sqr[:], sqt[:], 0.0, None, op0=OP.max)
                distt = small.tile([128, R], F32, tag=f"distt{c}")
                distt_l.append(distt)
                nc.scalar.activation(distt[:], sqr[:], AF.Sqrt)

                atcraw = work.tile([128, R], F32, tag="atcraw")
                nc.vector.tensor_tensor(atcraw[:], yownrep, ycb, op=OP.subtract)
                atc = small.tile([128, R], F32, tag=f"atc{c}")
                atc_l.append(atc)
                nc.scalar.activation(atc[:], atcraw[:], AF.Abs)
                samet = small.tile([128, R], F32, tag=f"samet{c}")
                samet_l.append(samet)
                nc.vector.tensor_tensor(samet[:], yownrep, ycb, op=OP.is_lt)
                ndt = small.tile([128, R], F32, tag=f"ndt{c}")
                ndt_l.append(ndt)
                nc.vector.tensor_tensor(
                    ndt[:], ownidxrep, jcolc[:, c:c + 1].to_broadcast((128, R)),
                    op=OP.not_equal,
                )

                # r-form thresholds: r_j = (y16_j - 2*y_i)*y16_j
                t_r = work.tile([128, R], F32, tag="t_r")
                nc.vector.scalar_tensor_tensor(
                    t_r[:], yownrep, -2.0, ycb16, op0=OP.mult, op1=OP.add
                )
                nc.vector.tensor_tensor(rtc16[:, csl], t_r[:], ycb16, op=OP.mult)
                nc.vector.tensor_copy(rtc32[:, csl], rtc16[:, csl])

            # ---------- pass 2: Sigmoid ----------
            dwt_l = []
            for c in range(CH):
                dwt = small.tile([128, R], F32, tag=f"dwt{c}")
                dwt_l.append(dwt)
                nc.scalar.activation(dwt[:], atc_l[c][:], AF.Sigmoid, scale=TAU)

            # ---------- pass 3: Exp + weights + row sums + g ----------
            cs_ps = ps_acc.tile([1, 3 * R], F32, tag="acc")
            for c in range(CH):
                csl = slice(c * R, (c + 1) * R)
                et = work.tile([128, R], F32, tag="et")
                nc.scalar.activation(et[:], distt_l[c][:], AF.Exp, scale=-1.0 / TEMP)
                wt = work.tile([128, R], F32, tag="wt")
                nc.vector.tensor_tensor(wt[:], et[:], dwt_l[c][:], op=OP.mult)
                csin = work.tile([128, 3 * R], F32, tag="csin")
                nc.vector.tensor_tensor(csin[:, 0:R], wt[:], ndt_l[c][:], op=OP.mult)
                nc.vector.tensor_tensor(
                    csin[:, R:2 * R], distt_l[c][:], ndt_l[c][:], op=OP.mult
                )
                # g = wd0 * ((POS_W-1+NEG_W)*same - NEG_W)
                ga = work.tile([128, R], F32, tag="ga")
                nc.vector.tensor_scalar(
                    ga[:], samet_l[c][:], POS_W - 1.0 + NEG_W, NEG_W,
                    op0=OP.mult, op1=OP.subtract,
                )
                nc.vector.tensor_tensor(gtc32[:, csl], ga[:], csin[:, 0:R], op=OP.mult)
                nc.vector.tensor_copy(csin[:, 2 * R:3 * R], gtc32[:, csl])
                nc.tensor.matmul(
                    cs_ps[:], ones128[:], csin[:], start=(c == 0), stop=(c == CH - 1)
                )

            cs_s = small.tile([1, 3 * R], F32, tag="cs_s")
            nc.vector.tensor_copy(cs_s[:], cs_ps[:])

            nrtc32 = small.tile([128, CH * R], F32, tag="nrtc32")
            nc.vector.tensor_scalar(nrtc32[:], rtc32[:], -1.0, None, op0=OP.mult)
            ghalf16 = small.tile([128, CH * R], F16, tag="ghalf16")
            nc.vector.tensor_scalar(ghalf16[:], gtc32[:], 0.5, None, op0=OP.mult)

            # ---------- crepquad: bias column per group ----------
            # Ccol[i] = C_i as a [R,1] column (PE transpose of the cs row)
            ccol_ps = ps_pre.tile([R, 1], F32, tag="pre")
            nc.tensor.matmul(
                ccol_ps[:], cs_s[0:1, 0:R], onesrow[0:1, 0:1], start=True, stop=True
            )
            ccol = small.tile([R, 1], F32, tag="ccol")
            nc.vector.tensor_copy(ccol[:], ccol_ps[:])
            # rhsEC[i,g] = C_i * [i//4 == g]
            rhs_ec = small.tile([R, NG], F32, tag="rhs_ec")
            nc.vector.tensor_scalar(rhs_ec[:], e_s, ccol[:, 0:1], None, op0=OP.mult)
            # crepquad[q,g] = sum_i SEL[i,q] * rhsEC[i,g] = C_{4g + q//32}
            cq_ps = ps_pre.tile([128, NG], F32, tag="pre")
            nc.tensor.matmul(cq_ps[:], sel_s, rhs_ec[:], start=True, stop=True)
            crepquad = small.tile([128, NG], F32, tag="crepquad")
            nc.vector.tensor_copy(crepquad[:], cq_ps[:])
            # sign-path rows need bias + GT_i/2 (their PSUM holds G - GT/2)
            gtcol_ps = ps_pre.tile([R, 1], F32, tag="pre")
            nc.tensor.matmul(
                gtcol_ps[:], cs_s[0:1, 2 * R:3 * R], onesrow[0:1, 0:1],
                start=True, stop=True,
            )
            gtcol = small.tile([R, 1], F32, tag="gtcol")
            nc.vector.tensor_copy(gtcol[:], gtcol_ps[:])
            rhs_gt = small.tile([R, NG], F32, tag="rhs_gt")
            nc.vector.tensor_scalar(rhs_gt[:], e2_s, gtcol[:, 0:1], None, op0=OP.mult)
            gt2_ps = ps_pre.tile([128, NG], F32, tag="pre")
            nc.tensor.matmul(gt2_ps[:], sel_s, rhs_gt[:], start=True, stop=True)
            nc.vector.scalar_tensor_tensor(
                crepquad[:], gt2_ps[:], 0.5, crepquad[:], op0=OP.mult, op1=OP.add
            )

            # ---------- main loop ----------
            lnaccg = small.tile([128, NG], F32, tag="lnaccg")
            for g in range(NG):
                gb = ps_g.tile([128, N], F32, tag="g")
                for k in range(GS):
                    i = GS * g + k
                    qrep = qall[:, i * N:(i + 1) * N]
                    thr = rtc32
                    for c in range(CH):
                        tso = tpool.tile([128, N], F16, tag="tso")
                        if i in SIGNI:
                            nc.scalar.activation(
                                tso[:], qrep, AF.Sign,
                                bias=nrtc32[:, c * R + i:c * R + i + 1],
                            )
                            lhs = ghalf16[:, c * R + i:c * R + i + 1].to_broadcast(
                                (128, 32)
                            )
                        else:
                            nc.vector.tensor_scalar(
                                tso[:], qrep,
                                thr[:, c * R + i:c * R + i + 1],
                                gtc32[:, c * R + i:c * R + i + 1],
                                op0=OP.is_gt, op1=OP.mult,
                            )
                            lhs = ones32h[:]
                        nc.tensor.matmul(
                            gb[32 * k:32 * k + 32, :], lhs, tso[:],
                            start=(c == 0),
                            stop=(c == CH - 1),
                            skip_group_check=True,
                        )
                lnout = lnpool.tile([128, N], F32, tag="lnout")
                nc.scalar.activation(
                    lnout[0:96, :], gb[0:96, :], AF.Ln,
                    bias=crepquad[0:96, g:g + 1],
                    accum_out=lnaccg[0:96, g:g + 1],
                )

            # ---------- outputs ----------
            ssum = small.tile([1, R], F32, tag="ssum")
            nc.vector.tensor_scalar(
                ssum[:], cs_s[0:1, R:2 * R], -1.0 / TEMP, None, op0=OP.mult
            )
            lnc = small.tile([1, R], F32, tag="lnc")
            nc.scalar.activation(lnc[:], cs_s[0:1, 0:R], AF.Ln)
            nc.sync.dma_start(out1[0:1, :], ssum[:])
            nc.sync.dma_start(out1[1:2, :], lnc[:])
            nc.sync.dma_start(out2, lnaccg[0:96, :])

    nc.compile()
    return nc


_NC_CACHE = None


def _get_nc():
    global _NC_CACHE
    if _NC_CACHE is None:
        _NC_CACHE = _build_program()
    return _NC_CACHE


def _make_in_maps(embeddings, targets):
    emb = np.ascontiguousarray(np.asarray(embeddings, dtype=np.float32))
    tgt = np.ascontiguousarray(np.asarray(targets, dtype=np.float32))
    z = emb.transpose(1, 0, 2).reshape(N, D)
    zT = np.ascontiguousarray(z.T)                       # [D, N]
    y = np.concatenate([tgt, tgt], axis=0)[:, 0]         # [N]
    jidx = np.arange(N, dtype=np.float32)
    # constant indicator matrices for the crepquad build
    sel = (np.arange(R)[:, None] % GS == np.arange(128)[None, :] // 32)
    emat = (np.arange(R)[:, None] // GS == np.arange(NG)[None, :])
    in_maps = []
    for core in range(NC):
        sl = slice(core * R, (core + 1) * R)
        p = np.zeros((128, PW), np.float32)
        p[:, C_ZT:C_ZT + N] = zT
        p[:, C_ZTOWN:C_ZTOWN + R] = zT[:, sl]
        p[:, C_YOWN:C_YOWN + R] = y[None, sl]
        p[:, C_OIDX:C_OIDX + R] = jidx[None, sl]
        p[:, C_YCOL:C_YCOL + CH] = y.reshape(CH, 128).T
        p[:, C_JCOL:C_JCOL + CH] = jidx.reshape(CH, 128).T
        p[:, C_YBC:C_YBC + N] = y[None, :]
        p[0:R, C_SEL:C_SEL + 128] = sel
        p[0:R, C_E:C_E + NG] = emat
        p[0:R, C_YOC] = y[sl]
        p[0:R, C_E2:C_E2 + NG] = emat * np.isin(np.arange(R), SIGNI)[:, None]
        in_maps.append({"packed": p})
    return in_maps


def _reduce_outs(outs_list):
    tot_s = 0.0
    tot_logd = 0.0
    ks = np.arange(R) % GS
    gs = np.arange(R) // GS
    for o in outs_list:
        o1 = np.asarray(o["out1"], dtype=np.float64)
        o2 = np.asarray(o["out2"], dtype=np.float64)
        tot_s += o1[0, :].sum()
        lnacc_i = o2[32 * ks, gs]        # [R] per-anchor sum_p ln(denom)
        tot_logd += (lnacc_i - o1[1, :]).sum()
    loss = -(tot_s - tot_logd) / (N * (N - 1))
    return np.float32(loss)


def _run(embeddings, targets, trace=False, **kw):
    nc = _get_nc()
    in_maps = _make_in_maps(embeddings, targets)
    res = run_bass_kernel_spmd(nc, in_maps, list(range(NC)), trace=trace, **kw)
    outs = [res.results[c] for c in range(NC)]
    return _reduce_outs(outs), res


def kernel(embeddings, targets):
    loss, _ = _run(embeddings, targets, trace=False)
    return loss
